# revision 12
# baseline (speedup 1.0000x reference)
"""Trainium2 Bass kernel for the NeuralMapCell problem.

Strategy (8 NeuronCores, SPMD):
  - conv_dense1 (238144x128, 122MB -- the dominant memory traffic) is sharded
    by rows (the contraction dim) across the 8 cores: core i holds the rows
    that multiply the flattened pool outputs of conv2-output-channels
    [8i, 8i+8).  Each core's 15.2MB chunk is streamed contiguously.
  - conv1 is input-channel sharded (32 of 256 channels per core); partial
    outputs (32x4096, 512KB) are AllReduced (overlaps the dense1 stream).
  - conv2/pool/flatten are output-channel sharded (8 of 64 channels/core).
  - The partial hidden vector h = flat @ d1 (128 floats) is AllReduced.
  - Attention (scores over the 4096-slot memory grid, softmax, read c_t) and
    the scalar bookkeeping are replicated on every core.
  - Each core emits rows [32i, 32i+32) of new_mem (the transposed memory grid
    with one column updated); the host concatenates the 8 chunks.

All heavy DRAM inputs are pre-tiled on the host so every big DMA is a fully
contiguous partition-major transfer.
"""

import numpy as np

import concourse.bass as bass
import concourse.mybir as mybir
import concourse.tile as tile
from concourse import bacc
from concourse import bass_utils

F32 = mybir.dt.float32


class _StageDone(Exception):
    pass

N_CORES = 8
UNITS = 256
H = W = 64
GRID = H * W                     # 4096
P61 = 61 * 61                    # 3721 pool positions per channel
CH_PER_CORE = 8                  # conv2 output channels per core
ROWS_PER_CORE = CH_PER_CORE * P61  # 29768 dense1 rows per core
NT = 30                          # 128-blocks per channel (3721 padded to 3840)
PPAD = NT * 128                  # 3840 padded positions per channel
NBLK = NT * CH_PER_CORE          # 240 K-blocks of 128
NCHUNK = 8                       # d1 DMA chunks
BLK_PER_CHUNK = NBLK // NCHUNK   # 30 blocks per chunk


def build_program(slot: int, stage: int = 99):
    """Trace the SPMD program (identical on all cores). Returns nc."""
    nc = bacc.Bacc(
        "TRN2", target_bir_lowering=False, debug=False,
        enable_asserts=True, num_devices=N_CORES,
    )

    # ---- DRAM I/O ----------------------------------------------------------
    def din(name, shape):
        return nc.dram_tensor(name, list(shape), F32, kind="ExternalInput").ap()

    x1s_d = din("x1s", (96, 4356))          # dx-stacked padded conv1 input slice
    k1_d = din("k1", (96, 96))              # conv1 weights [dx*32+ci, dy*32+co]
    k2_d = din("k2", (96, 72))              # conv2 wts [c1, (dy*3+dx)*8+co] x3
    id8_d = din("id8", (8, 8))
    id128_d = din("id128", (128, 128))
    inp_d = din("inp_col", (128, 1))        # inputs^T
    d1_d = din("d1", (NCHUNK, 128, BLK_PER_CHUNK * 128))   # tiled dense1 chunk
    d2_d = din("d2", (128, 256))
    ctx_d = din("ctxk", (128, 768))         # context kernel, partition-tiled
    wk_d = din("wk", (128, 256))            # write kernel (rec[:128])
    wu_d = din("wu", (128, 512))            # write update, partition-tiled
    wu32_d = din("wu32", (128, 64))         # write-update cols for my 32 out rows
    memtc_d = din("memt_col", (128, 2))     # memory[slot] as column halves
    memt32_d = din("memt32", (32, 1))       # memory[slot, 32i:32i+32]
    mem0_d = din("mem0", (128, 4096))       # memory rows 0..2048, tiled
    mem1_d = din("mem1", (128, 4096))       # memory rows 2048..4096, tiled

    ct_o = nc.dram_tensor("c_t", [1, 256], F32, kind="ExternalOutput").ap()
    rt_o = nc.dram_tensor("r_t", [1, 256], F32, kind="ExternalOutput").ap()
    nm_o = nc.dram_tensor("new_mem_chunk", [32, 4096], F32,
                          kind="ExternalOutput").ap()

    with tile.TileContext(nc) as tc:
      try:
        with (
            tc.tile_pool(name="big", bufs=7) as bigp,        # d1 chunks + mem halves
            tc.tile_pool(name="x1sp", bufs=1) as x1sp,
            tc.tile_pool(name="wts", bufs=1) as wts,
            tc.tile_pool(name="conv", bufs=1) as convp,
            tc.tile_pool(name="c1pool", bufs=2) as sharedp,   # c1 partial/full
            tc.tile_pool(name="c1spool", bufs=1) as c1sp,     # c1s <-> prod share
            tc.tile_pool(name="small", bufs=1) as smallp,
            tc.tile_pool(name="dram", bufs=1, space="DRAM") as dramp,
            tc.tile_pool(name="ps_cv", bufs=2, space="PSUM") as ps_cv,
            tc.tile_pool(name="ps_h", bufs=1, space="PSUM") as ps_h,
            tc.tile_pool(name="ps_t", bufs=2, space="PSUM") as ps_t,
        ):
            ps_c1 = ps_c2 = ps_cv
            ps_tr = ps_t
            # ---- early small loads (ACT ring: doesn't queue behind d1) ----
            x1s = x1sp.tile([96, 4356], F32, name="x1s_sb")
            nc.scalar.dma_start(out=x1s[:], in_=x1s_d)
            k1 = wts.tile([96, 96], F32, name="k1_sb")
            nc.scalar.dma_start(out=k1[:], in_=k1_d)
            k2 = wts.tile([96, 72], F32, name="k2_sb")
            nc.scalar.dma_start(out=k2[:], in_=k2_d)
            id8 = wts.tile([8, 8], F32, name="id8_sb")
            nc.scalar.dma_start(out=id8[:], in_=id8_d)
            id128 = wts.tile([128, 128], F32, name="id128_sb")
            nc.scalar.dma_start(out=id128[:], in_=id128_d)
            inp_col = wts.tile([128, 1], F32, name="inp_sb")
            nc.scalar.dma_start(out=inp_col[:], in_=inp_d)
            d2 = wts.tile([128, 256], F32, name="d2_sb")
            nc.scalar.dma_start(out=d2[:], in_=d2_d)
            ctxk = wts.tile([128, 768], F32, name="ctx_sb")
            nc.scalar.dma_start(out=ctxk[:], in_=ctx_d)
            wk = wts.tile([128, 256], F32, name="wk_sb")
            nc.scalar.dma_start(out=wk[:], in_=wk_d)
            wu = wts.tile([128, 512], F32, name="wu_sb")
            nc.scalar.dma_start(out=wu[:], in_=wu_d)
            wu32 = wts.tile([128, 64], F32, name="wu32_sb")
            nc.scalar.dma_start(out=wu32[:], in_=wu32_d)
            memtc = wts.tile([128, 2], F32, name="memtc_sb")
            nc.scalar.dma_start(out=memtc[:], in_=memtc_d)
            memt32 = wts.tile([32, 1], F32, name="memt32_sb")
            nc.scalar.dma_start(out=memt32[:], in_=memt32_d)
            # ---- the big streams (SP ring, strict FIFO order) -------------
            d1c = []
            for b in range(NCHUNK):
                t = bigp.tile([128, BLK_PER_CHUNK * 128], F32,
                              name=f"d1c{b}", tag="bigslot")
                nc.sync.dma_start(out=t[:], in_=d1_d[b])
                d1c.append(t)
            mem_t = []
            for hh in range(2):
                t = bigp.tile([128, 4096], F32, name=f"mem{hh}", tag="bigslot")
                nc.sync.dma_start(out=t[:], in_=(mem0_d if hh == 0 else mem1_d))
                mem_t.append(t)

            # ---- conv1 partial (K = 96 = 3dx * 32ci), SAME padding --------
            # Output is packed into a 3-partition-group halo layout:
            #   c1h[32g + c, 64*r + b] = conv1[c, input row GSTART[g]+r, b]
            # so conv2 (K=32) can read each output-row group from one
            # partition group with base partition in {0, 32, 64}.
            GSTART = [0, 22, 44]          # input-row start per group
            GLEN = [24, 24, 20]           # input rows held per group
            c1h_pre = sharedp.tile([96, 1536], F32, name="c1h_pre", tag="c1")
            nc.vector.memset(c1h_pre[64:96, 1280:1536], 0.0)
            x1s_v = x1s[:].rearrange("p (r q) -> p r q", q=66)
            for chunk in range(8):       # 8 rows of the 64x64 output at a time
                pt = ps_c1.tile([32, 512], F32, name="ps_c1t", tag="pscv")
                for dy in range(3):
                    nc.tensor.matmul(
                        pt[:].rearrange("p (a b) -> p a b", b=64),
                        k1[:, dy * 32:(dy + 1) * 32],
                        x1s_v[:, 8 * chunk + dy: 8 * chunk + dy + 8, 0:64],
                        start=(dy == 0), stop=(dy == 2),
                    )
                r_lo, r_hi = 8 * chunk, 8 * chunk + 8
                for g in range(3):
                    lo = max(r_lo, GSTART[g])
                    hi = min(r_hi, GSTART[g] + GLEN[g])
                    if lo >= hi:
                        continue
                    nc.vector.tensor_copy(
                        c1h_pre[32 * g:32 * (g + 1),
                                64 * (lo - GSTART[g]):64 * (hi - GSTART[g])],
                        pt[:, 64 * (lo - r_lo):64 * (hi - r_lo)])

            if stage < 1:
                raise _StageDone()
            # ---- AllReduce conv1 partials --------------------------------
            c1_in = dramp.tile([96, 1536], F32, name="c1_arin")
            c1_out = dramp.tile([96, 1536], F32, name="c1_arout")
            nc.gpsimd.dma_start(out=c1_in[:], in_=c1h_pre[:])
            nc.gpsimd.collective_compute(
                "AllReduce", mybir.AluOpType.add,
                replica_groups=[list(range(N_CORES))],
                ins=[c1_in[:].opt()], outs=[c1_out[:].opt()],
            )
            c1h = sharedp.tile([96, 1536], F32, name="c1h", tag="c1")
            nc.gpsimd.dma_start(out=c1h[:], in_=c1_out[:])

            if stage < 2:
                raise _StageDone()
            # ---- conv2 (VALID, 62x62 out, my 8 output channels, K=32) ----
            c2 = convp.tile([8, 62 * 62], F32, name="c2_sb")
            c1h_v = c1h[:].rearrange("p (r q) -> p r q", q=64)
            row_chunks = [(0, 8), (8, 8), (16, 6), (22, 8), (30, 8), (38, 6),
                          (44, 8), (52, 8), (60, 2)]
            for (r0, nr) in row_chunks:
                g = 0 if r0 < 22 else (1 if r0 < 44 else 2)
                rloc = r0 - GSTART[g]
                pt2 = ps_c2.tile([8, nr * 62], F32, name="ps_c2t", tag="pscv")
                kmm = 0
                for dy in range(3):
                    for dx in range(3):
                        nc.tensor.matmul(
                            pt2[:].rearrange("p (a b) -> p a b", b=62),
                            k2[32 * g:32 * (g + 1),
                               8 * (3 * dy + dx):8 * (3 * dy + dx + 1)],
                            c1h_v[32 * g:32 * (g + 1),
                                  rloc + dy: rloc + dy + nr, dx:dx + 62],
                            start=(kmm == 0), stop=(kmm == 8),
                        )
                        kmm += 1
                nc.vector.tensor_copy(c2[:, 62 * r0:62 * (r0 + nr)], pt2[:])

            # ---- 2x2 avg pool stride 1 (the 0.25 is folded into h) -------
            # padded to 3840 so the flat K-blocks are uniform 128s; the pad
            # region is zeroed (it multiplies zero rows of d1 anyway).
            pl = convp.tile([8, PPAD], F32, name="pl_sb")
            nc.vector.memset(pl[:, P61:PPAD], 0.0)
            c2v = c2[:].rearrange("p (a b) -> p a b", b=62)
            plv = pl[:, 0:P61].rearrange("p (a b) -> p a b", b=61)
            nc.vector.tensor_add(plv, c2v[:, 0:61, 0:61], c2v[:, 0:61, 1:62])
            nc.vector.tensor_add(plv, plv, c2v[:, 1:62, 0:61])
            nc.vector.tensor_add(plv, plv, c2v[:, 1:62, 1:62])

            # ---- transpose pool -> flat column blocks --------------------
            flat = convp.tile([128, NBLK], F32, name="flat_sb")
            # flat columns: j = t*8 + c  ->  flat[p, j] = pl[c, 128t + p]
            for t in range(NT):
                ptr = ps_tr.tile([128, 8], F32, name="ps_trt", tag="pst")
                nc.tensor.matmul(ptr[:], pl[:, 128 * t:128 * (t + 1)], id8[:],
                                 is_transpose=True)
                nc.vector.tensor_copy(flat[:, 8 * t:8 * (t + 1)], ptr[:])

            if stage < 3:
                raise _StageDone()
            # ---- dense1: h = 0.25 * flat @ d1 (128-dim, PSUM-accumulated) -
            ph = ps_h.tile([128, 1], F32, name="ps_h_t")
            for b in range(NCHUNK):
                for j in range(BLK_PER_CHUNK):
                    jj = b * BLK_PER_CHUNK + j
                    nc.tensor.matmul(
                        ph[:], d1c[b][:, 128 * j:128 * (j + 1)],
                        flat[:, jj:jj + 1],
                        start=(jj == 0), stop=(jj == NBLK - 1),
                    )
            h_sb = smallp.tile([128, 1], F32, name="h_sb")
            nc.scalar.mul(h_sb[:], ph[:], 0.25)

            if stage < 4:
                raise _StageDone()
            # ---- AllReduce h ---------------------------------------------
            h_in = dramp.tile([128, 1], F32, name="h_arin")
            h_out = dramp.tile([128, 1], F32, name="h_arout")
            nc.gpsimd.dma_start(out=h_in[:], in_=h_sb[:])
            nc.gpsimd.collective_compute(
                "AllReduce", mybir.AluOpType.add,
                replica_groups=[list(range(N_CORES))],
                ins=[h_in[:].opt()], outs=[h_out[:].opt()],
            )
            hf = smallp.tile([128, 1], F32, name="hf_sb")
            nc.gpsimd.dma_start(out=hf[:], in_=h_out[:])

            if stage < 5:
                raise _StageDone()
            # ---- r_t, q_t, s_t -------------------------------------------
            p_rt = ps_t.tile([1, 256], F32, name="p_rt", tag="pst")
            nc.tensor.matmul(p_rt[:], hf[:], d2[:], start=True, stop=True)
            rt = smallp.tile([1, 256], F32, name="rt_sb")
            nc.vector.tensor_copy(rt[:], p_rt[:])

            p_rtc = ps_t.tile([128, 2], F32, name="p_rtc", tag="pst")
            for hh in range(2):
                nc.tensor.matmul(p_rtc[:, hh:hh + 1],
                                 d2[:, 128 * hh:128 * (hh + 1)], hf[:],
                                 start=True, stop=True)
            rtc = smallp.tile([128, 2], F32, name="rtc_sb")
            nc.vector.tensor_copy(rtc[:], p_rtc[:])

            p_qt = ps_t.tile([1, 256], F32, name="p_qt", tag="pst")
            nc.tensor.matmul(p_qt[:], inp_col[:], ctxk[:, 0:256],
                             start=True, stop=False)
            nc.tensor.matmul(p_qt[:], rtc[:, 0:1], ctxk[:, 256:512],
                             start=False, stop=False)
            nc.tensor.matmul(p_qt[:], rtc[:, 1:2], ctxk[:, 512:768],
                             start=False, stop=True)
            qt = smallp.tile([1, 256], F32, name="qt_sb")
            nc.vector.tensor_copy(qt[:], p_qt[:])

            p_st = ps_t.tile([1, 256], F32, name="p_st", tag="pst")
            nc.tensor.matmul(p_st[:], inp_col[:], wk[:], start=True, stop=True)
            st = smallp.tile([1, 256], F32, name="st_sb")
            nc.vector.tensor_copy(st[:], p_st[:])

            p_stc = ps_t.tile([128, 2], F32, name="p_stc", tag="pst")
            for hh in range(2):
                nc.tensor.matmul(p_stc[:, hh:hh + 1],
                                 wk[:, 128 * hh:128 * (hh + 1)], inp_col[:],
                                 start=True, stop=True)
            stc = smallp.tile([128, 2], F32, name="stc_sb")
            nc.vector.tensor_copy(stc[:], p_stc[:])

            if stage < 6:
                raise _StageDone()
            # ---- scores = q_t @ memory^T over all 4096 grid slots --------
            sc = smallp.tile([128, 32], F32, name="sc_sb")
            prod = c1sp.tile([128, 2048], F32, name="prod", tag="c1s")
            qt_b = smallp.tile([128, 256], F32, name="qt_b")
            nc.gpsimd.partition_broadcast(qt_b[:], qt[:])
            qb = qt_b[:].rearrange("p (a c) -> p a c", a=1)
            qb = qb.broadcast_to((128, 8, 256))
            for g in range(4):
                mt = mem_t[g // 2]
                seg = mt[:, 2048 * (g % 2):2048 * (g % 2 + 1)]
                nc.vector.tensor_mul(
                    prod[:].rearrange("p (a b) -> p a b", b=256), seg
                    .rearrange("p (a b) -> p a b", b=256), qb)
                nc.vector.tensor_reduce(
                    sc[:, 8 * g:8 * (g + 1)],
                    prod[:].rearrange("p (a b) -> p a b", b=256),
                    axis=mybir.AxisListType.X, op=mybir.AluOpType.add)

            # ---- softmax (stable, denominator folded into c_t) -----------
            rmax = smallp.tile([128, 1], F32, name="rmax_sb")
            nc.vector.tensor_reduce(rmax[:], sc[:], axis=mybir.AxisListType.X,
                                    op=mybir.AluOpType.max)
            p_rm = ps_t.tile([1, 128], F32, name="p_rm", tag="pst")
            nc.tensor.matmul(p_rm[:], rmax[:], id128[:], is_transpose=True)
            rm_row = smallp.tile([1, 128], F32, name="rm_row")
            nc.vector.tensor_copy(rm_row[:], p_rm[:])
            gmax = smallp.tile([1, 1], F32, name="gmax_sb")
            nc.vector.tensor_reduce(gmax[:], rm_row[:],
                                    axis=mybir.AxisListType.X,
                                    op=mybir.AluOpType.max)
            gneg = smallp.tile([1, 1], F32, name="gneg_sb")
            nc.vector.tensor_scalar_mul(gneg[:], gmax[:], -1.0)

            gneg_b = smallp.tile([128, 1], F32, name="gneg_b")
            nc.gpsimd.partition_broadcast(gneg_b[:], gneg[:])
            ex = smallp.tile([128, 32], F32, name="ex_sb")
            rsum = smallp.tile([128, 1], F32, name="rsum_sb")
            nc.scalar.activation(ex[:], sc[:], mybir.ActivationFunctionType.Exp,
                                 bias=gneg_b[:], scale=1.0, accum_out=rsum[:])
            p_rs = ps_t.tile([1, 128], F32, name="p_rs", tag="pst")
            nc.tensor.matmul(p_rs[:], rsum[:], id128[:], is_transpose=True)
            rs_row = smallp.tile([1, 128], F32, name="rs_row")
            nc.vector.tensor_copy(rs_row[:], p_rs[:])
            gsum = smallp.tile([1, 1], F32, name="gsum_sb")
            nc.vector.tensor_reduce(gsum[:], rs_row[:],
                                    axis=mybir.AxisListType.X,
                                    op=mybir.AluOpType.add)
            ginv = smallp.tile([1, 1], F32, name="ginv_sb")
            nc.vector.reciprocal(ginv[:], gsum[:])

            if stage < 7:
                raise _StageDone()
            # ---- c_t = softmax(scores) @ memory --------------------------
            p_ct = ps_t.tile([1, 256], F32, name="p_ct", tag="pst")
            for j in range(32):
                mt = mem_t[j // 16]
                nc.tensor.matmul(p_ct[:], ex[:, j:j + 1],
                                 mt[:, 256 * (j % 16):256 * (j % 16 + 1)],
                                 start=(j == 0), stop=(j == 31))
            ct = smallp.tile([1, 256], F32, name="ct_sb")
            nc.scalar.mul(ct[:], p_ct[:], ginv[:])

            if stage < 8:
                raise _StageDone()
            # ---- importances, coef, memory-slot update -------------------
            scr = smallp.tile([1, 256], F32, name="scr_sb")
            scr2 = smallp.tile([1, 256], F32, name="scr2_sb")
            gimp = smallp.tile([1, 1], F32, name="gimp_sb")
            nc.vector.tensor_mul(scr[:], st[:], rt[:])
            nc.vector.tensor_reduce(gimp[:], scr[:], axis=mybir.AxisListType.X,
                                    op=mybir.AluOpType.add)
            limp = smallp.tile([1, 1], F32, name="limp_sb")
            nc.vector.tensor_mul(scr2[:], st[:], ct[:])
            nc.vector.tensor_reduce(limp[:], scr2[:], axis=mybir.AxisListType.X,
                                    op=mybir.AluOpType.add)
            den = smallp.tile([1, 1], F32, name="den_sb")
            nc.vector.tensor_add(den[:], gimp[:], limp[:])
            dinv = smallp.tile([1, 1], F32, name="dinv_sb")
            nc.vector.reciprocal(dinv[:], den[:])
            coef = smallp.tile([1, 1], F32, name="coef_sb")
            nc.vector.tensor_mul(coef[:], limp[:], dinv[:])

            if stage < 81:
                raise _StageDone()
            vcol = smallp.tile([128, 2], F32, name="vcol_sb")
            nc.vector.tensor_sub(vcol[:], memtc[:], stc[:])
            p_dm = ps_t.tile([32, 1], F32, name="p_dm", tag="pst")
            for hh in range(2):
                nc.tensor.matmul(p_dm[:], wu32[:, 32 * hh:32 * (hh + 1)],
                                 vcol[:, hh:hh + 1],
                                 start=(hh == 0), stop=(hh == 1))
            if stage < 82:
                raise _StageDone()
            coef_b = smallp.tile([32, 1], F32, name="coef_b")
            nc.gpsimd.partition_broadcast(coef_b[:], coef[:])
            d32 = smallp.tile([32, 1], F32, name="d32_sb")
            nc.vector.tensor_scalar_mul(d32[:], p_dm[:], coef_b[:])
            nv32 = smallp.tile([32, 1], F32, name="nv32_sb")
            nc.vector.tensor_add(nv32[:], memt32[:], d32[:])

            if stage < 83:
                raise _StageDone()
            nc.sync.dma_start(out=ct_o, in_=ct[:])
            nc.sync.dma_start(out=rt_o, in_=rt[:])
            if stage < 9:
                raise _StageDone()
            # patch the updated slot into x1s block d=1 (which holds my 32
            # rows of the grid at x1s[32+c, 66*(a+1)+b]), then write out
            pp = 66 * (slot // 64 + 1) + (slot % 64)
            nc.vector.tensor_copy(x1s[32:64, pp:pp + 1], nv32[:])
            x1s_3d = x1s[:].rearrange("p (r q) -> p r q", q=66)
            nc.sync.dma_start(out=nm_o.rearrange("p (a b) -> p a b", b=64),
                              in_=x1s_3d[32:64, 1:65, 0:64])
      except _StageDone:
        pass

    nc.compile()
    return nc


def shard_inputs(inputs, memory, conv_kernel1, conv_kernel2, conv_dense1,
                 conv_dense2, context_kernel, rec, slot):
    """Build the 8 per-core input dicts (host-side tiling/sharding)."""
    f32 = np.float32
    inputs = np.asarray(inputs, f32)
    memory = np.asarray(memory, f32)
    ck1 = np.asarray(conv_kernel1, f32)
    ck2 = np.asarray(conv_kernel2, f32)
    d1 = np.asarray(conv_dense1, f32)
    d2 = np.asarray(conv_dense2, f32)
    ctx = np.asarray(context_kernel, f32)
    rec = np.asarray(rec, f32)


    mem0 = np.ascontiguousarray(
        memory[:2048].reshape(16, 128, 256).transpose(1, 0, 2).reshape(128, 4096))
    mem1 = np.ascontiguousarray(
        memory[2048:].reshape(16, 128, 256).transpose(1, 0, 2).reshape(128, 4096))
    ctx_t = np.ascontiguousarray(
        ctx.reshape(3, 128, 256).transpose(1, 0, 2).reshape(128, 768))
    wu_t = np.ascontiguousarray(
        rec[128:].reshape(2, 128, 256).transpose(1, 0, 2).reshape(128, 512))
    id8 = np.eye(8, dtype=f32)
    id128 = np.eye(128, dtype=f32)
    inp_col = np.ascontiguousarray(inputs.reshape(128, 1))
    memt_col = np.ascontiguousarray(memory[slot].reshape(2, 128).T)

    in_maps = []
    for i in range(N_CORES):
        x1p = np.zeros((32, 66, 66), f32)
        x1p[:, 1:65, 1:65] = memory[:, 32 * i:32 * (i + 1)].T.reshape(32, 64, 64)
        x1p = x1p.reshape(32, 4356)
        x1s = np.zeros((96, 4356), f32)
        for d in range(3):
            x1s[d * 32:(d + 1) * 32, 0:4356 - d] = x1p[:, d:]

        k1 = np.ascontiguousarray(
            ck1[:, :, 32 * i:32 * (i + 1), :].transpose(1, 2, 0, 3).reshape(96, 96))
        k2one = ck2[:, :, :, 8 * i:8 * (i + 1)].transpose(2, 0, 1, 3).reshape(32, 72)
        k2 = np.ascontiguousarray(np.tile(k2one, (3, 1)))

        sl = d1[8 * i * P61:8 * (i + 1) * P61]          # (8*3721, 128)
        slp = np.zeros((CH_PER_CORE, PPAD, 128), f32)   # pad to 3840/channel
        slp[:, :P61] = sl.reshape(CH_PER_CORE, P61, 128)
        # device flat order: block j = t*8 + c holds positions 128t..128t+128
        # of channel c -> row (j*128 + p) = slp[c, 128t + p]
        slp = slp.reshape(CH_PER_CORE, NT, 128, 128).transpose(1, 0, 2, 3)
        d1main = np.ascontiguousarray(
            slp.reshape(NCHUNK, BLK_PER_CHUNK, 128, 128)
            .transpose(0, 2, 1, 3).reshape(NCHUNK, 128, BLK_PER_CHUNK * 128))

        wu32 = np.ascontiguousarray(
            rec[128:].reshape(2, 128, 256)[:, :, 32 * i:32 * (i + 1)]
            .transpose(1, 0, 2).reshape(128, 64))
        memt32 = np.ascontiguousarray(
            memory[slot, 32 * i:32 * (i + 1)].reshape(32, 1))

        in_maps.append({
            "x1s": x1s, "k1": k1, "k2": k2, "id8": id8, "id128": id128,
            "inp_col": inp_col, "d1": d1main,
            "d2": d2, "ctxk": ctx_t, "wk": np.ascontiguousarray(rec[:128]),
            "wu": wu_t, "wu32": wu32, "memt_col": memt_col, "memt32": memt32,
            "mem0": mem0, "mem1": mem1,
        })
    return in_maps


_CACHE = {}


def _install_ntff_hook():
    """Register the axon NTFF profiling hook that this image's boot skips
    (its `antenv` package lacks `axon_hooks`). Needed only for trace runs."""
    import sys
    import types
    if "antenv.axon_hooks" in sys.modules:
        return
    mod = types.ModuleType("antenv.axon_hooks")
    mod._hook = None

    def set_axon_ntff_profile_hook(h):
        mod._hook = h

    def get_axon_ntff_profile_hook():
        return mod._hook

    mod.set_axon_ntff_profile_hook = set_axon_ntff_profile_hook
    mod.get_axon_ntff_profile_hook = get_axon_ntff_profile_hook
    sys.modules["antenv.axon_hooks"] = mod
    try:
        import antenv
        antenv.axon_hooks = mod
    except Exception:
        pass
    try:
        from trn_agent_boot.trn_boot import _ntff_profile_via_ctypes
        mod.set_axon_ntff_profile_hook(
            _ntff_profile_via_ctypes("/opt/axon/libaxon_pjrt.so"))
    except Exception:
        pass
    # the trace path tries to upload artifacts to S3; keep it local
    bass_utils.upload_artifacts = lambda d: str(d)


def kernel(inputs, memory, conv_kernel1, conv_kernel2, conv_dense1,
           conv_dense2, context_kernel, recurrent_kernel=None,
           recurr_kernel=None, x=32, y=17, _want_trace=False):
    rec = recurr_kernel if recurr_kernel is not None else recurrent_kernel
    slot = int(x) * 64 + int(y)

    if slot not in _CACHE:
        _CACHE[slot] = build_program(slot)
    nc = _CACHE[slot]

    in_maps = shard_inputs(inputs, memory, conv_kernel1, conv_kernel2,
                           conv_dense1, conv_dense2, context_kernel, rec, slot)

    if _want_trace:
        _install_ntff_hook()
    res = bass_utils.run_bass_kernel_spmd(
        nc, in_maps, core_ids=list(range(N_CORES)), trace=_want_trace,
    )
    c_t = np.asarray(res.results[0]["c_t"])
    r_t = np.asarray(res.results[0]["r_t"])
    new_mem = np.concatenate(
        [np.asarray(res.results[i]["new_mem_chunk"]) for i in range(N_CORES)],
        axis=0).reshape(UNITS, H, W)
    out = (c_t, r_t, new_mem)
    if _want_trace:
        return out, res
    return out


# revision 18
# speedup vs baseline: 1.1234x; 1.1234x over previous
"""Trainium2 Bass kernel for the NeuralMapCell problem.

Strategy (8 NeuronCores, SPMD):
  - conv_dense1 (238144x128, 122MB -- the dominant memory traffic) is sharded
    by rows (the contraction dim) across the 8 cores: core i holds the rows
    that multiply the flattened pool outputs of conv2-output-channels
    [8i, 8i+8).  Each core's 15.2MB chunk is streamed contiguously.
  - conv1 is input-channel sharded (32 of 256 channels per core); partial
    outputs (32x4096, 512KB) are AllReduced (overlaps the dense1 stream).
  - conv2/pool/flatten are output-channel sharded (8 of 64 channels/core).
  - The partial hidden vector h = flat @ d1 (128 floats) is AllReduced.
  - Attention (scores over the 4096-slot memory grid, softmax, read c_t) and
    the scalar bookkeeping are replicated on every core.
  - Each core emits rows [32i, 32i+32) of new_mem (the transposed memory grid
    with one column updated); the host concatenates the 8 chunks.

All heavy DRAM inputs are pre-tiled on the host so every big DMA is a fully
contiguous partition-major transfer.
"""

import numpy as np

import concourse.bass as bass
import concourse.mybir as mybir
import concourse.tile as tile
from concourse import bacc
from concourse import bass_utils

F32 = mybir.dt.float32


class _StageDone(Exception):
    pass

N_CORES = 8
UNITS = 256
H = W = 64
GRID = H * W                     # 4096
P61 = 61 * 61                    # 3721 pool positions per channel
CH_PER_CORE = 8                  # conv2 output channels per core
ROWS_PER_CORE = CH_PER_CORE * P61  # 29768 dense1 rows per core
NT = 30                          # 128-blocks per channel (3721 padded to 3840)
PPAD = NT * 128                  # 3840 padded positions per channel
NBLK = NT * CH_PER_CORE          # 240 K-blocks of 128
NCHUNK = 8                       # d1 DMA chunks
BLK_PER_CHUNK = NBLK // NCHUNK   # 30 blocks per chunk


def build_program(slot: int, stage: int = 99):
    """Trace the SPMD program (identical on all cores). Returns nc."""
    nc = bacc.Bacc(
        "TRN2", target_bir_lowering=False, debug=False,
        enable_asserts=True, num_devices=N_CORES,
    )

    # ---- DRAM I/O ----------------------------------------------------------
    def din(name, shape):
        return nc.dram_tensor(name, list(shape), F32, kind="ExternalInput").ap()

    x1s_d = din("x1s", (96, 4356))          # dx-stacked padded conv1 input slice
    k1_d = din("k1", (96, 96))              # conv1 weights [dx*32+ci, dy*32+co]
    k2_d = din("k2", (96, 24))              # conv2 weights [dx*32+c1, dy*8+co]
    id8_d = din("id8", (8, 8))
    id1_d = din("id1", (1, 1))
    ones_d = din("ones_col", (128, 1))
    id128_d = din("id128", (128, 128))
    inp_d = din("inp_col", (128, 1))        # inputs^T
    d1_d = din("d1", (NCHUNK, 128, BLK_PER_CHUNK * 128))   # tiled dense1 chunk
    d2_d = din("d2", (128, 256))
    ctx_d = din("ctxk", (128, 768))         # context kernel, partition-tiled
    wk_d = din("wk", (128, 256))            # write kernel (rec[:128])
    wu_d = din("wu", (128, 512))            # write update, partition-tiled
    wu32_d = din("wu32", (128, 64))         # write-update cols for my 32 out rows
    memtc_d = din("memt_col", (128, 2))     # memory[slot] as column halves
    memt32_d = din("memt32", (32, 1))       # memory[slot, 32i:32i+32]
    mem0_d = din("mem0", (128, 4096))       # memory rows 0..2048, tiled
    mem1_d = din("mem1", (128, 4096))       # memory rows 2048..4096, tiled

    ct_o = nc.dram_tensor("c_t", [1, 256], F32, kind="ExternalOutput").ap()
    rt_o = nc.dram_tensor("r_t", [1, 256], F32, kind="ExternalOutput").ap()
    nm_o = nc.dram_tensor("new_mem_chunk", [32, 4096], F32,
                          kind="ExternalOutput").ap()

    with tile.TileContext(nc) as tc:
      try:
        with (
            tc.tile_pool(name="big", bufs=6) as bigp,        # d1 chunks + mem halves
            tc.tile_pool(name="x1sp", bufs=1) as x1sp,
            tc.tile_pool(name="wts", bufs=1) as wts,
            tc.tile_pool(name="conv", bufs=1) as convp,
            tc.tile_pool(name="c1pool", bufs=2) as sharedp,   # c1 partial/full
            tc.tile_pool(name="c1spool", bufs=1) as c1sp,     # c1s <-> prod share
            tc.tile_pool(name="small", bufs=1) as smallp,
            tc.tile_pool(name="dram", bufs=1, space="DRAM") as dramp,
            tc.tile_pool(name="ps_cv", bufs=2, space="PSUM") as ps_cv,
            tc.tile_pool(name="ps_h", bufs=1, space="PSUM") as ps_h,
            tc.tile_pool(name="ps_t", bufs=2, space="PSUM") as ps_t,
        ):
            ps_c1 = ps_c2 = ps_cv
            ps_tr = ps_t
            # ---- early small loads (ACT ring: doesn't queue behind d1) ----
            x1s = x1sp.tile([96, 4356], F32, name="x1s_sb")
            nc.scalar.dma_start(out=x1s[:], in_=x1s_d)
            k1 = wts.tile([96, 96], F32, name="k1_sb")
            nc.scalar.dma_start(out=k1[:], in_=k1_d)
            k2 = wts.tile([96, 24], F32, name="k2_sb")
            nc.scalar.dma_start(out=k2[:], in_=k2_d)
            id8 = wts.tile([8, 8], F32, name="id8_sb")
            nc.scalar.dma_start(out=id8[:], in_=id8_d)
            id1 = wts.tile([1, 1], F32, name="id1_sb")
            nc.scalar.dma_start(out=id1[:], in_=id1_d)
            ones_col = wts.tile([128, 1], F32, name="ones_sb")
            nc.scalar.dma_start(out=ones_col[:], in_=ones_d)
            id128 = wts.tile([128, 128], F32, name="id128_sb")
            nc.scalar.dma_start(out=id128[:], in_=id128_d)
            inp_col = wts.tile([128, 1], F32, name="inp_sb")
            nc.scalar.dma_start(out=inp_col[:], in_=inp_d)
            d2 = wts.tile([128, 256], F32, name="d2_sb")
            nc.scalar.dma_start(out=d2[:], in_=d2_d)
            ctxk = wts.tile([128, 768], F32, name="ctx_sb")
            nc.scalar.dma_start(out=ctxk[:], in_=ctx_d)
            wk = wts.tile([128, 256], F32, name="wk_sb")
            nc.scalar.dma_start(out=wk[:], in_=wk_d)
            wu = wts.tile([128, 512], F32, name="wu_sb")
            nc.scalar.dma_start(out=wu[:], in_=wu_d)
            wu32 = wts.tile([128, 64], F32, name="wu32_sb")
            nc.scalar.dma_start(out=wu32[:], in_=wu32_d)
            memtc = wts.tile([128, 2], F32, name="memtc_sb")
            nc.scalar.dma_start(out=memtc[:], in_=memtc_d)
            memt32 = wts.tile([32, 1], F32, name="memt32_sb")
            nc.scalar.dma_start(out=memt32[:], in_=memt32_d)
            # ---- the big streams (SP ring, strict FIFO order) -------------
            d1c = []
            for b in range(NCHUNK):
                t = bigp.tile([128, BLK_PER_CHUNK * 128], F32,
                              name=f"d1c{b}", tag="bigslot")
                nc.sync.dma_start(out=t[:], in_=d1_d[b])
                d1c.append(t)
            mem_t = []
            for hh in range(2):
                t = bigp.tile([128, 4096], F32, name=f"mem{hh}", tag="bigslot")
                nc.sync.dma_start(out=t[:], in_=(mem0_d if hh == 0 else mem1_d))
                mem_t.append(t)

            # ---- PE warmup: ~4us of junk matmuls so HAM unthrottles -------
            ps_warm = ps_h.tile([128, 128], F32, name="ps_warm", tag="warm")
            for w in range(8):
                nc.tensor.matmul(ps_warm[:], id128[:], id128[:],
                                 start=True, stop=True)

            # ---- conv1 partial (K = 96 = 3dx * 32ci), SAME padding --------
            # Output is packed into a 3-partition-group halo layout:
            #   c1h[32g + c, 64*r + b] = conv1[c, input row GSTART[g]+r, b]
            # so conv2 (K=32) can read each output-row group from one
            # partition group with base partition in {0, 32, 64}.
            GSTART = [0, 22, 44]          # input-row start per group
            GLEN = [24, 24, 20]           # input rows held per group
            c1h_pre = sharedp.tile([96, 1536], F32, name="c1h_pre", tag="c1")
            nc.vector.memset(c1h_pre[64:96, 1280:1536], 0.0)
            x1s_v = x1s[:].rearrange("p (r q) -> p r q", q=66)
            for chunk in range(8):       # 8 rows of the 64x64 output at a time
                pt = ps_c1.tile([32, 512], F32, name="ps_c1t", tag="pscv")
                for dy in range(3):
                    nc.tensor.matmul(
                        pt[:].rearrange("p (a b) -> p a b", b=64),
                        k1[:, dy * 32:(dy + 1) * 32],
                        x1s_v[:, 8 * chunk + dy: 8 * chunk + dy + 8, 0:64],
                        start=(dy == 0), stop=(dy == 2),
                    )
                r_lo, r_hi = 8 * chunk, 8 * chunk + 8
                for g in range(3):
                    lo = max(r_lo, GSTART[g])
                    hi = min(r_hi, GSTART[g] + GLEN[g])
                    if lo >= hi:
                        continue
                    nc.vector.tensor_copy(
                        c1h_pre[32 * g:32 * (g + 1),
                                64 * (lo - GSTART[g]):64 * (hi - GSTART[g])],
                        pt[:, 64 * (lo - r_lo):64 * (hi - r_lo)])

            if stage < 1:
                raise _StageDone()
            # ---- AllReduce conv1 partials --------------------------------
            c1_in = dramp.tile([96, 1536], F32, name="c1_arin")
            c1_out = dramp.tile([96, 1536], F32, name="c1_arout")
            nc.gpsimd.dma_start(out=c1_in[:], in_=c1h_pre[:])
            nc.gpsimd.collective_compute(
                "AllReduce", mybir.AluOpType.add,
                replica_groups=[list(range(N_CORES))],
                ins=[c1_in[:].opt()], outs=[c1_out[:].opt()],
            )
            c1h = sharedp.tile([96, 1536], F32, name="c1h", tag="c1")
            nc.gpsimd.dma_start(out=c1h[:], in_=c1_out[:])

            if stage < 2:
                raise _StageDone()
            # ---- rebuild the dx-stacked conv1 output from the halo groups -
            # c1s[d*32+c, p] = conv1[c, p+d]; group g holds conv1 positions
            # [64*GSTART[g], 64*(GSTART[g]+GLEN[g])) at local q = pos - 64*GS.
            c1s = c1sp.tile([96, 4096], F32, name="c1s", tag="c1s")
            gseg = [(0, 0, 1536), (1, 1536, 2944), (2, 2944, 4096)]
            for d in range(3):
                for g, lo, hi in gseg:
                    slo = max(lo, d)            # source position range [slo, hi)
                    loc = slo - 64 * GSTART[g]  # local offset inside group g
                    nc.vector.tensor_copy(
                        c1s[d * 32:(d + 1) * 32, slo - d:hi - d],
                        c1h[32 * g:32 * (g + 1), loc:loc + (hi - slo)])

            # ---- conv2 (VALID, 62x62 out, my 8 output channels, K=96) ----
            c2 = convp.tile([8, 62 * 62], F32, name="c2_sb")
            c1s_v = c1s[:].rearrange("p (r q) -> p r q", q=64)
            row_chunks = [(0, 8), (8, 8), (16, 8), (24, 8),
                          (32, 8), (40, 8), (48, 8), (56, 6)]
            for (r0, nr) in row_chunks:
                pt2 = ps_c2.tile([8, nr * 62], F32, name="ps_c2t", tag="pscv")
                for dy in range(3):
                    nc.tensor.matmul(
                        pt2[:].rearrange("p (a b) -> p a b", b=62),
                        k2[:, dy * 8:(dy + 1) * 8],
                        c1s_v[:, r0 + dy: r0 + dy + nr, 0:62],
                        start=(dy == 0), stop=(dy == 2),
                    )
                nc.vector.tensor_copy(c2[:, 62 * r0:62 * (r0 + nr)], pt2[:])

            # ---- 2x2 avg pool stride 1 (the 0.25 is folded into h) -------
            # padded to 3840 so the flat K-blocks are uniform 128s; the pad
            # region is zeroed (it multiplies zero rows of d1 anyway).
            # Emitted in 8-row steps so transposes/dense1 pipeline behind
            # conv2 instead of waiting for the full pool.
            pl = convp.tile([8, PPAD], F32, name="pl_sb")
            nc.vector.memset(pl[:, P61:PPAD], 0.0)
            c2v = c2[:].rearrange("p (a b) -> p a b", b=62)
            plv = pl[:, 0:P61].rearrange("p (a b) -> p a b", b=61)
            for r0 in range(0, 61, 8):
                r1 = min(r0 + 8, 61)
                nc.vector.tensor_add(plv[:, r0:r1, :],
                                     c2v[:, r0:r1, 0:61], c2v[:, r0:r1, 1:62])
                nc.vector.tensor_add(plv[:, r0:r1, :], plv[:, r0:r1, :],
                                     c2v[:, r0 + 1:r1 + 1, 0:61])
                nc.vector.tensor_add(plv[:, r0:r1, :], plv[:, r0:r1, :],
                                     c2v[:, r0 + 1:r1 + 1, 1:62])

            # ---- transpose pool -> flat column blocks --------------------
            flat = convp.tile([128, NBLK], F32, name="flat_sb")
            # flat columns: j = t*8 + c  ->  flat[p, j] = pl[c, 128t + p]
            for t in range(NT):
                ptr = ps_tr.tile([128, 8], F32, name="ps_trt", tag="pst")
                nc.tensor.matmul(ptr[:], pl[:, 128 * t:128 * (t + 1)], id8[:],
                                 is_transpose=True)
                nc.vector.tensor_copy(flat[:, 8 * t:8 * (t + 1)], ptr[:])

            if stage < 3:
                raise _StageDone()
            # ---- dense1 on the vector engine ------------------------------
            # d1c[b] is laid out [p, m*30 + j]; multiply by flat[p, 30b + j]
            # (broadcast over m), reduce over j, and accumulate the
            # per-partition partials; a final ones-matmul sums partitions.
            hacc = convp.tile([128, 128], F32, name="hacc_sb")
            hred = convp.tile([128, 128], F32, name="hred_sb")
            JH = BLK_PER_CHUNK // 2
            prod = convp.tile([128, 128 * JH], F32, name="prodd")
            first = True
            for b in range(NCHUNK):
                d3 = d1c[b][:].rearrange("p (m j) -> p m j", j=BLK_PER_CHUNK)
                for jh in range(2):
                    fb = flat[:, 30 * b + JH * jh:30 * b + JH * (jh + 1)]
                    fbb = fb.rearrange("p (a j) -> p a j", a=1)
                    fbb = fbb.broadcast_to((128, 128, JH))
                    p3 = prod[:].rearrange("p (m j) -> p m j", j=JH)
                    nc.vector.tensor_mul(
                        p3, d3[:, :, JH * jh:JH * (jh + 1)], fbb)
                    if first:
                        nc.vector.tensor_reduce(hacc[:], p3,
                                                axis=mybir.AxisListType.X,
                                                op=mybir.AluOpType.add)
                        first = False
                    else:
                        nc.vector.tensor_reduce(hred[:], p3,
                                                axis=mybir.AxisListType.X,
                                                op=mybir.AluOpType.add)
                        nc.vector.tensor_add(hacc[:], hacc[:], hred[:])
            ph = ps_h.tile([1, 128], F32, name="ps_h_t")
            nc.tensor.matmul(ph[:], ones_col[:], hacc[:], start=True, stop=True)
            hrow = smallp.tile([1, 128], F32, name="hrow_sb")
            nc.vector.tensor_copy(hrow[:], ph[:])
            pht = ps_h.tile([128, 1], F32, name="ps_ht", tag="ps_h_t")
            nc.tensor.matmul(pht[:], hrow[:], id1[:], is_transpose=True)
            h_sb = smallp.tile([128, 1], F32, name="h_sb")
            nc.scalar.mul(h_sb[:], pht[:], 0.25)

            if stage < 4:
                raise _StageDone()
            # ---- AllReduce h ---------------------------------------------
            h_in = dramp.tile([128, 1], F32, name="h_arin")
            h_out = dramp.tile([128, 1], F32, name="h_arout")
            nc.gpsimd.dma_start(out=h_in[:], in_=h_sb[:])
            nc.gpsimd.collective_compute(
                "AllReduce", mybir.AluOpType.add,
                replica_groups=[list(range(N_CORES))],
                ins=[h_in[:].opt()], outs=[h_out[:].opt()],
            )
            hf = smallp.tile([128, 1], F32, name="hf_sb")
            nc.gpsimd.dma_start(out=hf[:], in_=h_out[:])

            if stage < 5:
                raise _StageDone()
            # ---- r_t, q_t, s_t -------------------------------------------
            p_rt = ps_t.tile([1, 256], F32, name="p_rt", tag="pst")
            nc.tensor.matmul(p_rt[:], hf[:], d2[:], start=True, stop=True)
            rt = smallp.tile([1, 256], F32, name="rt_sb")
            nc.vector.tensor_copy(rt[:], p_rt[:])

            p_rtc = ps_t.tile([128, 2], F32, name="p_rtc", tag="pst")
            for hh in range(2):
                nc.tensor.matmul(p_rtc[:, hh:hh + 1],
                                 d2[:, 128 * hh:128 * (hh + 1)], hf[:],
                                 start=True, stop=True)
            rtc = smallp.tile([128, 2], F32, name="rtc_sb")
            nc.vector.tensor_copy(rtc[:], p_rtc[:])

            p_qt = ps_t.tile([1, 256], F32, name="p_qt", tag="pst")
            nc.tensor.matmul(p_qt[:], inp_col[:], ctxk[:, 0:256],
                             start=True, stop=False)
            nc.tensor.matmul(p_qt[:], rtc[:, 0:1], ctxk[:, 256:512],
                             start=False, stop=False)
            nc.tensor.matmul(p_qt[:], rtc[:, 1:2], ctxk[:, 512:768],
                             start=False, stop=True)
            qt = smallp.tile([1, 256], F32, name="qt_sb")
            nc.vector.tensor_copy(qt[:], p_qt[:])

            p_st = ps_t.tile([1, 256], F32, name="p_st", tag="pst")
            nc.tensor.matmul(p_st[:], inp_col[:], wk[:], start=True, stop=True)
            st = smallp.tile([1, 256], F32, name="st_sb")
            nc.vector.tensor_copy(st[:], p_st[:])

            p_stc = ps_t.tile([128, 2], F32, name="p_stc", tag="pst")
            for hh in range(2):
                nc.tensor.matmul(p_stc[:, hh:hh + 1],
                                 wk[:, 128 * hh:128 * (hh + 1)], inp_col[:],
                                 start=True, stop=True)
            stc = smallp.tile([128, 2], F32, name="stc_sb")
            nc.vector.tensor_copy(stc[:], p_stc[:])

            if stage < 6:
                raise _StageDone()
            # ---- scores = q_t @ memory^T over all 4096 grid slots --------
            sc = smallp.tile([128, 32], F32, name="sc_sb")
            prod = c1sp.tile([128, 2048], F32, name="prod", tag="c1s")
            qt_b = smallp.tile([128, 256], F32, name="qt_b")
            nc.gpsimd.partition_broadcast(qt_b[:], qt[:])
            qb = qt_b[:].rearrange("p (a c) -> p a c", a=1)
            qb = qb.broadcast_to((128, 8, 256))
            for g in range(4):
                mt = mem_t[g // 2]
                seg = mt[:, 2048 * (g % 2):2048 * (g % 2 + 1)]
                nc.vector.tensor_mul(
                    prod[:].rearrange("p (a b) -> p a b", b=256), seg
                    .rearrange("p (a b) -> p a b", b=256), qb)
                nc.vector.tensor_reduce(
                    sc[:, 8 * g:8 * (g + 1)],
                    prod[:].rearrange("p (a b) -> p a b", b=256),
                    axis=mybir.AxisListType.X, op=mybir.AluOpType.add)

            # ---- softmax (stable, denominator folded into c_t) -----------
            rmax = smallp.tile([128, 1], F32, name="rmax_sb")
            nc.vector.tensor_reduce(rmax[:], sc[:], axis=mybir.AxisListType.X,
                                    op=mybir.AluOpType.max)
            p_rm = ps_t.tile([1, 128], F32, name="p_rm", tag="pst")
            nc.tensor.matmul(p_rm[:], rmax[:], id128[:], is_transpose=True)
            rm_row = smallp.tile([1, 128], F32, name="rm_row")
            nc.vector.tensor_copy(rm_row[:], p_rm[:])
            gmax = smallp.tile([1, 1], F32, name="gmax_sb")
            nc.vector.tensor_reduce(gmax[:], rm_row[:],
                                    axis=mybir.AxisListType.X,
                                    op=mybir.AluOpType.max)
            gneg = smallp.tile([1, 1], F32, name="gneg_sb")
            nc.vector.tensor_scalar_mul(gneg[:], gmax[:], -1.0)

            gneg_b = smallp.tile([128, 1], F32, name="gneg_b")
            nc.gpsimd.partition_broadcast(gneg_b[:], gneg[:])
            ex = smallp.tile([128, 32], F32, name="ex_sb")
            rsum = smallp.tile([128, 1], F32, name="rsum_sb")
            nc.scalar.activation(ex[:], sc[:], mybir.ActivationFunctionType.Exp,
                                 bias=gneg_b[:], scale=1.0, accum_out=rsum[:])
            p_rs = ps_t.tile([1, 128], F32, name="p_rs", tag="pst")
            nc.tensor.matmul(p_rs[:], rsum[:], id128[:], is_transpose=True)
            rs_row = smallp.tile([1, 128], F32, name="rs_row")
            nc.vector.tensor_copy(rs_row[:], p_rs[:])
            gsum = smallp.tile([1, 1], F32, name="gsum_sb")
            nc.vector.tensor_reduce(gsum[:], rs_row[:],
                                    axis=mybir.AxisListType.X,
                                    op=mybir.AluOpType.add)
            ginv = smallp.tile([1, 1], F32, name="ginv_sb")
            nc.vector.reciprocal(ginv[:], gsum[:])

            if stage < 7:
                raise _StageDone()
            # ---- c_t = softmax(scores) @ memory --------------------------
            p_ct = ps_t.tile([1, 256], F32, name="p_ct", tag="pst")
            for j in range(32):
                mt = mem_t[j // 16]
                nc.tensor.matmul(p_ct[:], ex[:, j:j + 1],
                                 mt[:, 256 * (j % 16):256 * (j % 16 + 1)],
                                 start=(j == 0), stop=(j == 31))
            ct = smallp.tile([1, 256], F32, name="ct_sb")
            nc.scalar.mul(ct[:], p_ct[:], ginv[:])

            if stage < 8:
                raise _StageDone()
            # ---- importances, coef, memory-slot update -------------------
            scr = smallp.tile([1, 256], F32, name="scr_sb")

            gimp = smallp.tile([1, 1], F32, name="gimp_sb")
            nc.vector.tensor_mul(scr[:], st[:], rt[:])
            nc.vector.tensor_reduce(gimp[:], scr[:], axis=mybir.AxisListType.X,
                                    op=mybir.AluOpType.add)
            limp = smallp.tile([1, 1], F32, name="limp_sb")
            nc.vector.tensor_mul(scr[:], st[:], ct[:])
            nc.vector.tensor_reduce(limp[:], scr[:], axis=mybir.AxisListType.X,
                                    op=mybir.AluOpType.add)
            den = smallp.tile([1, 1], F32, name="den_sb")
            nc.vector.tensor_add(den[:], gimp[:], limp[:])
            dinv = smallp.tile([1, 1], F32, name="dinv_sb")
            nc.vector.reciprocal(dinv[:], den[:])
            coef = smallp.tile([1, 1], F32, name="coef_sb")
            nc.vector.tensor_mul(coef[:], limp[:], dinv[:])

            if stage < 81:
                raise _StageDone()
            vcol = smallp.tile([128, 2], F32, name="vcol_sb")
            nc.vector.tensor_sub(vcol[:], memtc[:], stc[:])
            p_dm = ps_t.tile([32, 1], F32, name="p_dm", tag="pst")
            for hh in range(2):
                nc.tensor.matmul(p_dm[:], wu32[:, 32 * hh:32 * (hh + 1)],
                                 vcol[:, hh:hh + 1],
                                 start=(hh == 0), stop=(hh == 1))
            if stage < 82:
                raise _StageDone()
            coef_b = smallp.tile([32, 1], F32, name="coef_b")
            nc.gpsimd.partition_broadcast(coef_b[:], coef[:])
            d32 = smallp.tile([32, 1], F32, name="d32_sb")
            nc.vector.tensor_scalar_mul(d32[:], p_dm[:], coef_b[:])
            nv32 = smallp.tile([32, 1], F32, name="nv32_sb")
            nc.vector.tensor_add(nv32[:], memt32[:], d32[:])

            if stage < 83:
                raise _StageDone()
            nc.sync.dma_start(out=ct_o, in_=ct[:])
            nc.sync.dma_start(out=rt_o, in_=rt[:])
            if stage < 9:
                raise _StageDone()
            # patch the updated slot into x1s block d=1 (which holds my 32
            # rows of the grid at x1s[32+c, 66*(a+1)+b]), then write out
            pp = 66 * (slot // 64 + 1) + (slot % 64)
            nc.vector.tensor_copy(x1s[32:64, pp:pp + 1], nv32[:])
            x1s_3d = x1s[:].rearrange("p (r q) -> p r q", q=66)
            nc.sync.dma_start(out=nm_o.rearrange("p (a b) -> p a b", b=64),
                              in_=x1s_3d[32:64, 1:65, 0:64])
      except _StageDone:
        pass

    nc.compile()
    return nc


def shard_inputs(inputs, memory, conv_kernel1, conv_kernel2, conv_dense1,
                 conv_dense2, context_kernel, rec, slot):
    """Build the 8 per-core input dicts (host-side tiling/sharding)."""
    f32 = np.float32
    inputs = np.asarray(inputs, f32)
    memory = np.asarray(memory, f32)
    ck1 = np.asarray(conv_kernel1, f32)
    ck2 = np.asarray(conv_kernel2, f32)
    d1 = np.asarray(conv_dense1, f32)
    d2 = np.asarray(conv_dense2, f32)
    ctx = np.asarray(context_kernel, f32)
    rec = np.asarray(rec, f32)


    mem0 = np.ascontiguousarray(
        memory[:2048].reshape(16, 128, 256).transpose(1, 0, 2).reshape(128, 4096))
    mem1 = np.ascontiguousarray(
        memory[2048:].reshape(16, 128, 256).transpose(1, 0, 2).reshape(128, 4096))
    ctx_t = np.ascontiguousarray(
        ctx.reshape(3, 128, 256).transpose(1, 0, 2).reshape(128, 768))
    wu_t = np.ascontiguousarray(
        rec[128:].reshape(2, 128, 256).transpose(1, 0, 2).reshape(128, 512))
    id8 = np.eye(8, dtype=f32)
    id1 = np.ones((1, 1), f32)
    ones_col = np.ones((128, 1), f32)
    id128 = np.eye(128, dtype=f32)
    inp_col = np.ascontiguousarray(inputs.reshape(128, 1))
    memt_col = np.ascontiguousarray(memory[slot].reshape(2, 128).T)

    in_maps = []
    for i in range(N_CORES):
        x1p = np.zeros((32, 66, 66), f32)
        x1p[:, 1:65, 1:65] = memory[:, 32 * i:32 * (i + 1)].T.reshape(32, 64, 64)
        x1p = x1p.reshape(32, 4356)
        x1s = np.zeros((96, 4356), f32)
        for d in range(3):
            x1s[d * 32:(d + 1) * 32, 0:4356 - d] = x1p[:, d:]

        k1 = np.ascontiguousarray(
            ck1[:, :, 32 * i:32 * (i + 1), :].transpose(1, 2, 0, 3).reshape(96, 96))
        k2 = np.ascontiguousarray(
            ck2[:, :, :, 8 * i:8 * (i + 1)].transpose(1, 2, 0, 3).reshape(96, 24))

        sl = d1[8 * i * P61:8 * (i + 1) * P61]          # (8*3721, 128)
        slp = np.zeros((CH_PER_CORE, PPAD, 128), f32)   # pad to 3840/channel
        slp[:, :P61] = sl.reshape(CH_PER_CORE, P61, 128)
        # flat block j = t*8 + c holds positions 128t..128t+128 of channel c;
        # device layout is [chunk][p, m*30 + j_local] so the DVE can multiply
        # rows by flat values and reduce over the innermost j axis.
        slp = slp.reshape(CH_PER_CORE, NT, 128, 128).transpose(1, 0, 2, 3)
        d1main = np.ascontiguousarray(
            slp.reshape(NCHUNK, BLK_PER_CHUNK, 128, 128)
            .transpose(0, 2, 3, 1).reshape(NCHUNK, 128, 128 * BLK_PER_CHUNK))

        wu32 = np.ascontiguousarray(
            rec[128:].reshape(2, 128, 256)[:, :, 32 * i:32 * (i + 1)]
            .transpose(1, 0, 2).reshape(128, 64))
        memt32 = np.ascontiguousarray(
            memory[slot, 32 * i:32 * (i + 1)].reshape(32, 1))

        in_maps.append({
            "x1s": x1s, "k1": k1, "k2": k2, "id8": id8, "id128": id128,
            "id1": id1, "ones_col": ones_col,
            "inp_col": inp_col, "d1": d1main,
            "d2": d2, "ctxk": ctx_t, "wk": np.ascontiguousarray(rec[:128]),
            "wu": wu_t, "wu32": wu32, "memt_col": memt_col, "memt32": memt32,
            "mem0": mem0, "mem1": mem1,
        })
    return in_maps


_CACHE = {}


def _install_ntff_hook():
    """Register the axon NTFF profiling hook that this image's boot skips
    (its `antenv` package lacks `axon_hooks`). Needed only for trace runs."""
    import sys
    import types
    if "antenv.axon_hooks" in sys.modules:
        return
    mod = types.ModuleType("antenv.axon_hooks")
    mod._hook = None

    def set_axon_ntff_profile_hook(h):
        mod._hook = h

    def get_axon_ntff_profile_hook():
        return mod._hook

    mod.set_axon_ntff_profile_hook = set_axon_ntff_profile_hook
    mod.get_axon_ntff_profile_hook = get_axon_ntff_profile_hook
    sys.modules["antenv.axon_hooks"] = mod
    try:
        import antenv
        antenv.axon_hooks = mod
    except Exception:
        pass
    try:
        from trn_agent_boot.trn_boot import _ntff_profile_via_ctypes
        mod.set_axon_ntff_profile_hook(
            _ntff_profile_via_ctypes("/opt/axon/libaxon_pjrt.so"))
    except Exception:
        pass
    # the trace path tries to upload artifacts to S3; keep it local
    bass_utils.upload_artifacts = lambda d: str(d)


def kernel(inputs, memory, conv_kernel1, conv_kernel2, conv_dense1,
           conv_dense2, context_kernel, recurrent_kernel=None,
           recurr_kernel=None, x=32, y=17, _want_trace=False):
    rec = recurr_kernel if recurr_kernel is not None else recurrent_kernel
    slot = int(x) * 64 + int(y)

    if slot not in _CACHE:
        _CACHE[slot] = build_program(slot)
    nc = _CACHE[slot]

    in_maps = shard_inputs(inputs, memory, conv_kernel1, conv_kernel2,
                           conv_dense1, conv_dense2, context_kernel, rec, slot)

    if _want_trace:
        _install_ntff_hook()
    res = bass_utils.run_bass_kernel_spmd(
        nc, in_maps, core_ids=list(range(N_CORES)), trace=_want_trace,
    )
    c_t = np.asarray(res.results[0]["c_t"])
    r_t = np.asarray(res.results[0]["r_t"])
    new_mem = np.concatenate(
        [np.asarray(res.results[i]["new_mem_chunk"]) for i in range(N_CORES)],
        axis=0).reshape(UNITS, H, W)
    out = (c_t, r_t, new_mem)
    if _want_trace:
        return out, res
    return out


# revision 20
# speedup vs baseline: 1.4943x; 1.3301x over previous
"""Trainium2 Bass kernel for the NeuralMapCell problem.

Strategy (8 NeuronCores, SPMD):
  - conv_dense1 (238144x128, 122MB -- the dominant memory traffic) is sharded
    by rows (the contraction dim) across the 8 cores: core i holds the rows
    that multiply the flattened pool outputs of conv2-output-channels
    [8i, 8i+8).  Each core's 15.2MB chunk is streamed contiguously.
  - conv1 is input-channel sharded (32 of 256 channels per core); partial
    outputs (32x4096, 512KB) are AllReduced (overlaps the dense1 stream).
  - conv2/pool/flatten are output-channel sharded (8 of 64 channels/core).
  - The partial hidden vector h = flat @ d1 (128 floats) is AllReduced.
  - Attention (scores over the 4096-slot memory grid, softmax, read c_t) and
    the scalar bookkeeping are replicated on every core.
  - Each core emits rows [32i, 32i+32) of new_mem (the transposed memory grid
    with one column updated); the host concatenates the 8 chunks.

All heavy DRAM inputs are pre-tiled on the host so every big DMA is a fully
contiguous partition-major transfer.
"""

import numpy as np

import concourse.bass as bass
import concourse.mybir as mybir
import concourse.tile as tile
from concourse import bacc
from concourse import bass_utils

F32 = mybir.dt.float32


class _StageDone(Exception):
    pass

N_CORES = 8
UNITS = 256
H = W = 64
GRID = H * W                     # 4096
P61 = 61 * 61                    # 3721 pool positions per channel
CH_PER_CORE = 8                  # conv2 output channels per core
ROWS_PER_CORE = CH_PER_CORE * P61  # 29768 dense1 rows per core
NT = 30                          # 128-blocks per channel (3721 padded to 3840)
PPAD = NT * 128                  # 3840 padded positions per channel
NBLK = NT * CH_PER_CORE          # 240 K-blocks of 128
NCHUNK = 8                       # d1 DMA chunks
BLK_PER_CHUNK = NBLK // NCHUNK   # 30 blocks per chunk


def build_program(slot: int, stage: int = 99):
    """Trace the SPMD program (identical on all cores). Returns nc."""
    nc = bacc.Bacc(
        "TRN2", target_bir_lowering=False, debug=False,
        enable_asserts=True, num_devices=N_CORES,
    )

    # ---- DRAM I/O ----------------------------------------------------------
    def din(name, shape):
        return nc.dram_tensor(name, list(shape), F32, kind="ExternalInput").ap()

    x1s_d = din("x1s", (96, 4356))          # dx-stacked padded conv1 input slice
    k1_d = din("k1", (96, 96))              # conv1 weights [dx*32+ci, dy*32+co]
    k2_d = din("k2", (96, 24))              # conv2 weights [dx*32+c1, dy*8+co]
    id8_d = din("id8", (8, 8))
    id1_d = din("id1", (1, 1))
    ones_d = din("ones_col", (128, 1))
    id128_d = din("id128", (128, 128))
    inp_d = din("inp_col", (128, 1))        # inputs^T
    d1_d = din("d1", (NBLK // 4, 128, 512))   # diagonal-batched dense1
    d2_d = din("d2", (128, 256))
    ctx_d = din("ctxk", (128, 768))         # context kernel, partition-tiled
    wk_d = din("wk", (128, 256))            # write kernel (rec[:128])
    wu_d = din("wu", (128, 512))            # write update, partition-tiled
    wu32_d = din("wu32", (128, 64))         # write-update cols for my 32 out rows
    memtc_d = din("memt_col", (128, 2))     # memory[slot] as column halves
    memt32_d = din("memt32", (32, 1))       # memory[slot, 32i:32i+32]
    mem0_d = din("mem0", (128, 4096))       # memory rows 0..2048, tiled
    mem1_d = din("mem1", (128, 4096))       # memory rows 2048..4096, tiled

    ct_o = nc.dram_tensor("c_t", [1, 256], F32, kind="ExternalOutput").ap()
    rt_o = nc.dram_tensor("r_t", [1, 256], F32, kind="ExternalOutput").ap()
    nm_o = nc.dram_tensor("new_mem_chunk", [32, 4096], F32,
                          kind="ExternalOutput").ap()

    with tile.TileContext(nc) as tc:
      try:
        with (
            tc.tile_pool(name="big", bufs=6) as bigp,        # d1 chunks + mem halves
            tc.tile_pool(name="x1sp", bufs=1) as x1sp,
            tc.tile_pool(name="wts", bufs=1) as wts,
            tc.tile_pool(name="conv", bufs=1) as convp,
            tc.tile_pool(name="c1pool", bufs=2) as sharedp,   # c1 partial/full
            tc.tile_pool(name="c1spool", bufs=1) as c1sp,     # c1s <-> prod share
            tc.tile_pool(name="small", bufs=1) as smallp,
            tc.tile_pool(name="dram", bufs=1, space="DRAM") as dramp,
            tc.tile_pool(name="ps_cv", bufs=2, space="PSUM") as ps_cv,
            tc.tile_pool(name="ps_h", bufs=1, space="PSUM") as ps_h,
            tc.tile_pool(name="ps_t", bufs=2, space="PSUM") as ps_t,
        ):
            ps_c1 = ps_c2 = ps_cv
            ps_tr = ps_t
            # ---- dummy collective: absorbs the cross-core launch-skew
            # barrier + ncfw cold start concurrently with the DMA phase ----
            du_in = dramp.tile([1, 128], F32, name="du_in")
            du_out = dramp.tile([1, 128], F32, name="du_out")
            nc.gpsimd.collective_compute(
                "AllReduce", mybir.AluOpType.add,
                replica_groups=[list(range(N_CORES))],
                ins=[du_in[:].opt()], outs=[du_out[:].opt()])

            # ---- early small loads (ACT ring: doesn't queue behind d1) ----
            x1s = x1sp.tile([96, 4356], F32, name="x1s_sb")
            nc.scalar.dma_start(out=x1s[:], in_=x1s_d)
            k1 = wts.tile([96, 96], F32, name="k1_sb")
            nc.scalar.dma_start(out=k1[:], in_=k1_d)
            k2 = wts.tile([96, 24], F32, name="k2_sb")
            nc.scalar.dma_start(out=k2[:], in_=k2_d)
            id8 = wts.tile([8, 8], F32, name="id8_sb")
            nc.scalar.dma_start(out=id8[:], in_=id8_d)
            id1 = wts.tile([1, 1], F32, name="id1_sb")
            nc.scalar.dma_start(out=id1[:], in_=id1_d)
            ones_col = wts.tile([128, 1], F32, name="ones_sb")
            nc.scalar.dma_start(out=ones_col[:], in_=ones_d)
            id128 = wts.tile([128, 128], F32, name="id128_sb")
            nc.scalar.dma_start(out=id128[:], in_=id128_d)
            inp_col = wts.tile([128, 1], F32, name="inp_sb")
            nc.scalar.dma_start(out=inp_col[:], in_=inp_d)
            d2 = wts.tile([128, 256], F32, name="d2_sb")
            nc.scalar.dma_start(out=d2[:], in_=d2_d)
            ctxk = wts.tile([128, 768], F32, name="ctx_sb")
            nc.scalar.dma_start(out=ctxk[:], in_=ctx_d)
            wk = wts.tile([128, 256], F32, name="wk_sb")
            nc.scalar.dma_start(out=wk[:], in_=wk_d)
            wu = wts.tile([128, 512], F32, name="wu_sb")
            nc.scalar.dma_start(out=wu[:], in_=wu_d)
            wu32 = wts.tile([128, 64], F32, name="wu32_sb")
            nc.scalar.dma_start(out=wu32[:], in_=wu32_d)
            memtc = wts.tile([128, 2], F32, name="memtc_sb")
            nc.scalar.dma_start(out=memtc[:], in_=memtc_d)
            memt32 = wts.tile([32, 1], F32, name="memt32_sb")
            nc.scalar.dma_start(out=memt32[:], in_=memt32_d)
            # ---- the big streams (SP ring, strict FIFO order) -------------
            # 60 groups of 512 cols -> 7 chunks of 8 groups + 1 of 4
            d1c = []
            gpc = []
            for b in range(NCHUNK):
                g0, g1 = 8 * b, min(8 * (b + 1), NBLK // 4)
                t = bigp.tile([128, 512 * (g1 - g0)], F32,
                              name=f"d1c{b}", tag="bigslot")
                nc.sync.dma_start(
                    out=t[:].rearrange("p (g n) -> p g n", n=512),
                    in_=d1_d[g0:g1].rearrange("g p n -> p g n"))
                d1c.append(t)
                gpc.append((g0, g1))
            mem_t = []
            for hh in range(2):
                t = bigp.tile([128, 4096], F32, name=f"mem{hh}", tag="bigslot")
                nc.sync.dma_start(out=t[:], in_=(mem0_d if hh == 0 else mem1_d))
                mem_t.append(t)

            # ---- PE warmup: ~4us of junk matmuls so HAM unthrottles -------
            ps_warm = ps_h.tile([128, 128], F32, name="ps_warm", tag="warm")
            for w in range(8):
                nc.tensor.matmul(ps_warm[:], id128[:], id128[:],
                                 start=True, stop=True)

            # ---- conv1 partial (K = 96 = 3dx * 32ci), SAME padding --------
            # Output is packed into a 3-partition-group halo layout:
            #   c1h[32g + c, 64*r + b] = conv1[c, input row GSTART[g]+r, b]
            # so conv2 (K=32) can read each output-row group from one
            # partition group with base partition in {0, 32, 64}.
            GSTART = [0, 22, 44]          # input-row start per group
            GLEN = [24, 24, 20]           # input rows held per group
            c1h_pre = sharedp.tile([96, 1536], F32, name="c1h_pre", tag="c1")
            nc.vector.memset(c1h_pre[64:96, 1280:1536], 0.0)
            x1s_v = x1s[:].rearrange("p (r q) -> p r q", q=66)
            for chunk in range(8):       # 8 rows of the 64x64 output at a time
                pt = ps_c1.tile([32, 512], F32, name="ps_c1t", tag="pscv")
                for dy in range(3):
                    nc.tensor.matmul(
                        pt[:].rearrange("p (a b) -> p a b", b=64),
                        k1[:, dy * 32:(dy + 1) * 32],
                        x1s_v[:, 8 * chunk + dy: 8 * chunk + dy + 8, 0:64],
                        start=(dy == 0), stop=(dy == 2),
                    )
                r_lo, r_hi = 8 * chunk, 8 * chunk + 8
                for g in range(3):
                    lo = max(r_lo, GSTART[g])
                    hi = min(r_hi, GSTART[g] + GLEN[g])
                    if lo >= hi:
                        continue
                    nc.vector.tensor_copy(
                        c1h_pre[32 * g:32 * (g + 1),
                                64 * (lo - GSTART[g]):64 * (hi - GSTART[g])],
                        pt[:, 64 * (lo - r_lo):64 * (hi - r_lo)])

            if stage < 1:
                raise _StageDone()
            # ---- AllReduce conv1 partials --------------------------------
            c1_in = dramp.tile([96, 1536], F32, name="c1_arin")
            c1_out = dramp.tile([96, 1536], F32, name="c1_arout")
            nc.gpsimd.dma_start(out=c1_in[:], in_=c1h_pre[:])
            nc.gpsimd.collective_compute(
                "AllReduce", mybir.AluOpType.add,
                replica_groups=[list(range(N_CORES))],
                ins=[c1_in[:].opt()], outs=[c1_out[:].opt()],
            )
            c1h = sharedp.tile([96, 1536], F32, name="c1h", tag="c1")
            nc.gpsimd.dma_start(out=c1h[:], in_=c1_out[:])

            if stage < 2:
                raise _StageDone()
            # ---- rebuild the dx-stacked conv1 output from the halo groups -
            # c1s[d*32+c, p] = conv1[c, p+d]; group g holds conv1 positions
            # [64*GSTART[g], 64*(GSTART[g]+GLEN[g])) at local q = pos - 64*GS.
            c1s = c1sp.tile([96, 4096], F32, name="c1s", tag="c1s")
            gseg = [(0, 0, 1536), (1, 1536, 2944), (2, 2944, 4096)]
            for d in range(3):
                for g, lo, hi in gseg:
                    slo = max(lo, d)            # source position range [slo, hi)
                    loc = slo - 64 * GSTART[g]  # local offset inside group g
                    nc.vector.tensor_copy(
                        c1s[d * 32:(d + 1) * 32, slo - d:hi - d],
                        c1h[32 * g:32 * (g + 1), loc:loc + (hi - slo)])

            # ---- conv2 (VALID, 62x62 out, my 8 output channels, K=96) ----
            c2 = convp.tile([8, 62 * 62], F32, name="c2_sb")
            c1s_v = c1s[:].rearrange("p (r q) -> p r q", q=64)
            row_chunks = [(0, 8), (8, 8), (16, 8), (24, 8),
                          (32, 8), (40, 8), (48, 8), (56, 6)]
            for (r0, nr) in row_chunks:
                pt2 = ps_c2.tile([8, nr * 62], F32, name="ps_c2t", tag="pscv")
                for dy in range(3):
                    nc.tensor.matmul(
                        pt2[:].rearrange("p (a b) -> p a b", b=62),
                        k2[:, dy * 8:(dy + 1) * 8],
                        c1s_v[:, r0 + dy: r0 + dy + nr, 0:62],
                        start=(dy == 0), stop=(dy == 2),
                    )
                nc.vector.tensor_copy(c2[:, 62 * r0:62 * (r0 + nr)], pt2[:])

            # ---- 2x2 avg pool stride 1 (the 0.25 is folded into h) -------
            # padded to 3840 so the flat K-blocks are uniform 128s; the pad
            # region is zeroed (it multiplies zero rows of d1 anyway).
            # Emitted in 8-row steps so transposes/dense1 pipeline behind
            # conv2 instead of waiting for the full pool.
            pl = convp.tile([8, PPAD], F32, name="pl_sb")
            nc.vector.memset(pl[:, P61:PPAD], 0.0)
            c2v = c2[:].rearrange("p (a b) -> p a b", b=62)
            plv = pl[:, 0:P61].rearrange("p (a b) -> p a b", b=61)
            for r0 in range(0, 61, 8):
                r1 = min(r0 + 8, 61)
                nc.vector.tensor_add(plv[:, r0:r1, :],
                                     c2v[:, r0:r1, 0:61], c2v[:, r0:r1, 1:62])
                nc.vector.tensor_add(plv[:, r0:r1, :], plv[:, r0:r1, :],
                                     c2v[:, r0 + 1:r1 + 1, 0:61])
                nc.vector.tensor_add(plv[:, r0:r1, :], plv[:, r0:r1, :],
                                     c2v[:, r0 + 1:r1 + 1, 1:62])

            # ---- transpose pool -> flat column blocks --------------------
            flat = convp.tile([128, NBLK], F32, name="flat_sb")
            # flat columns: j = t*8 + c  ->  flat[p, j] = pl[c, 128t + p]
            for t in range(NT):
                ptr = ps_tr.tile([128, 8], F32, name="ps_trt", tag="pst")
                nc.tensor.matmul(ptr[:], pl[:, 128 * t:128 * (t + 1)], id8[:],
                                 is_transpose=True)
                nc.vector.tensor_copy(flat[:, 8 * t:8 * (t + 1)], ptr[:])

            if stage < 3:
                raise _StageDone()
            # ---- dense1: diagonal-batched GEMM on the PE ------------------
            # matmul g: lhsT = flat[:, 4g:4g+4], rhs = 4 packed d1 blocks
            # (128,512); useful results accumulate on out[c, 128c+m].
            ph4 = ps_h.tile([4, 512], F32, name="ps_h4")
            ng = NBLK // 4
            for b in range(NCHUNK):
                g0, g1 = gpc[b]
                for g in range(g0, g1):
                    nc.tensor.matmul(
                        ph4[:], flat[:, 4 * g:4 * (g + 1)],
                        d1c[b][:, 512 * (g - g0):512 * (g - g0 + 1)],
                        start=(g == 0), stop=(g == ng - 1))
            # extract the diagonal: transpose each 128-col slice of the
            # (4,512) accumulator, then sum the free-dim diagonal (stride 5).
            sb4 = smallp.tile([4, 512], F32, name="sb4_sb")
            nc.vector.tensor_copy(sb4[:], ph4[:])
            ht4 = smallp.tile([128, 16], F32, name="ht4_sb")
            for c in range(4):
                ptc = ps_t.tile([128, 4], F32, name="ps_htc", tag="pst")
                nc.tensor.matmul(ptc[:], sb4[:, 128 * c:128 * (c + 1)],
                                 id8[0:4, 0:4], is_transpose=True)
                nc.vector.tensor_copy(ht4[:, 4 * c:4 * (c + 1)], ptc[:])
            import dataclasses as _dc
            htv = ht4[:]
            diag = _dc.replace(htv, ap=[htv.ap[0], [5, 4]])
            h_sb = smallp.tile([128, 1], F32, name="h_sb")
            nc.vector.tensor_reduce(h_sb[:], diag, axis=mybir.AxisListType.X,
                                    op=mybir.AluOpType.add)
            nc.vector.tensor_scalar_mul(h_sb[:], h_sb[:], 0.25)

            if stage < 4:
                raise _StageDone()
            # ---- AllReduce h ---------------------------------------------
            h_in = dramp.tile([128, 1], F32, name="h_arin")
            h_out = dramp.tile([128, 1], F32, name="h_arout")
            nc.gpsimd.dma_start(out=h_in[:], in_=h_sb[:])
            nc.gpsimd.collective_compute(
                "AllReduce", mybir.AluOpType.add,
                replica_groups=[list(range(N_CORES))],
                ins=[h_in[:].opt()], outs=[h_out[:].opt()],
            )
            hf = smallp.tile([128, 1], F32, name="hf_sb")
            nc.gpsimd.dma_start(out=hf[:], in_=h_out[:])

            if stage < 5:
                raise _StageDone()
            # ---- r_t, q_t, s_t -------------------------------------------
            p_rt = ps_t.tile([1, 256], F32, name="p_rt", tag="pst")
            nc.tensor.matmul(p_rt[:], hf[:], d2[:], start=True, stop=True)
            rt = smallp.tile([1, 256], F32, name="rt_sb")
            nc.vector.tensor_copy(rt[:], p_rt[:])

            p_rtc = ps_t.tile([128, 2], F32, name="p_rtc", tag="pst")
            for hh in range(2):
                nc.tensor.matmul(p_rtc[:, hh:hh + 1],
                                 d2[:, 128 * hh:128 * (hh + 1)], hf[:],
                                 start=True, stop=True)
            rtc = smallp.tile([128, 2], F32, name="rtc_sb")
            nc.vector.tensor_copy(rtc[:], p_rtc[:])

            p_qt = ps_t.tile([1, 256], F32, name="p_qt", tag="pst")
            nc.tensor.matmul(p_qt[:], inp_col[:], ctxk[:, 0:256],
                             start=True, stop=False)
            nc.tensor.matmul(p_qt[:], rtc[:, 0:1], ctxk[:, 256:512],
                             start=False, stop=False)
            nc.tensor.matmul(p_qt[:], rtc[:, 1:2], ctxk[:, 512:768],
                             start=False, stop=True)
            qt = smallp.tile([1, 256], F32, name="qt_sb")
            nc.vector.tensor_copy(qt[:], p_qt[:])

            p_st = ps_t.tile([1, 256], F32, name="p_st", tag="pst")
            nc.tensor.matmul(p_st[:], inp_col[:], wk[:], start=True, stop=True)
            st = smallp.tile([1, 256], F32, name="st_sb")
            nc.vector.tensor_copy(st[:], p_st[:])

            p_stc = ps_t.tile([128, 2], F32, name="p_stc", tag="pst")
            for hh in range(2):
                nc.tensor.matmul(p_stc[:, hh:hh + 1],
                                 wk[:, 128 * hh:128 * (hh + 1)], inp_col[:],
                                 start=True, stop=True)
            stc = smallp.tile([128, 2], F32, name="stc_sb")
            nc.vector.tensor_copy(stc[:], p_stc[:])

            if stage < 6:
                raise _StageDone()
            # ---- scores = q_t @ memory^T over all 4096 grid slots --------
            sc = smallp.tile([128, 32], F32, name="sc_sb")
            prod = c1sp.tile([128, 2048], F32, name="prod", tag="c1s")
            qt_b = smallp.tile([128, 256], F32, name="qt_b")
            nc.gpsimd.partition_broadcast(qt_b[:], qt[:])
            qb = qt_b[:].rearrange("p (a c) -> p a c", a=1)
            qb = qb.broadcast_to((128, 8, 256))
            for g in range(4):
                mt = mem_t[g // 2]
                seg = mt[:, 2048 * (g % 2):2048 * (g % 2 + 1)]
                nc.vector.tensor_mul(
                    prod[:].rearrange("p (a b) -> p a b", b=256), seg
                    .rearrange("p (a b) -> p a b", b=256), qb)
                nc.vector.tensor_reduce(
                    sc[:, 8 * g:8 * (g + 1)],
                    prod[:].rearrange("p (a b) -> p a b", b=256),
                    axis=mybir.AxisListType.X, op=mybir.AluOpType.add)

            # ---- softmax (stable, denominator folded into c_t) -----------
            rmax = smallp.tile([128, 1], F32, name="rmax_sb")
            nc.vector.tensor_reduce(rmax[:], sc[:], axis=mybir.AxisListType.X,
                                    op=mybir.AluOpType.max)
            p_rm = ps_t.tile([1, 128], F32, name="p_rm", tag="pst")
            nc.tensor.matmul(p_rm[:], rmax[:], id128[:], is_transpose=True)
            rm_row = smallp.tile([1, 128], F32, name="rm_row")
            nc.vector.tensor_copy(rm_row[:], p_rm[:])
            gmax = smallp.tile([1, 1], F32, name="gmax_sb")
            nc.vector.tensor_reduce(gmax[:], rm_row[:],
                                    axis=mybir.AxisListType.X,
                                    op=mybir.AluOpType.max)
            gneg = smallp.tile([1, 1], F32, name="gneg_sb")
            nc.vector.tensor_scalar_mul(gneg[:], gmax[:], -1.0)

            gneg_b = smallp.tile([128, 1], F32, name="gneg_b")
            nc.gpsimd.partition_broadcast(gneg_b[:], gneg[:])
            ex = smallp.tile([128, 32], F32, name="ex_sb")
            rsum = smallp.tile([128, 1], F32, name="rsum_sb")
            nc.scalar.activation(ex[:], sc[:], mybir.ActivationFunctionType.Exp,
                                 bias=gneg_b[:], scale=1.0, accum_out=rsum[:])
            p_rs = ps_t.tile([1, 128], F32, name="p_rs", tag="pst")
            nc.tensor.matmul(p_rs[:], rsum[:], id128[:], is_transpose=True)
            rs_row = smallp.tile([1, 128], F32, name="rs_row")
            nc.vector.tensor_copy(rs_row[:], p_rs[:])
            gsum = smallp.tile([1, 1], F32, name="gsum_sb")
            nc.vector.tensor_reduce(gsum[:], rs_row[:],
                                    axis=mybir.AxisListType.X,
                                    op=mybir.AluOpType.add)
            ginv = smallp.tile([1, 1], F32, name="ginv_sb")
            nc.vector.reciprocal(ginv[:], gsum[:])

            if stage < 7:
                raise _StageDone()
            # ---- c_t = softmax(scores) @ memory --------------------------
            p_ct = ps_t.tile([1, 256], F32, name="p_ct", tag="pst")
            for j in range(32):
                mt = mem_t[j // 16]
                nc.tensor.matmul(p_ct[:], ex[:, j:j + 1],
                                 mt[:, 256 * (j % 16):256 * (j % 16 + 1)],
                                 start=(j == 0), stop=(j == 31))
            ct = smallp.tile([1, 256], F32, name="ct_sb")
            nc.scalar.mul(ct[:], p_ct[:], ginv[:])

            if stage < 8:
                raise _StageDone()
            # ---- importances, coef, memory-slot update -------------------
            scr = smallp.tile([1, 256], F32, name="scr_sb")

            gimp = smallp.tile([1, 1], F32, name="gimp_sb")
            nc.vector.tensor_mul(scr[:], st[:], rt[:])
            nc.vector.tensor_reduce(gimp[:], scr[:], axis=mybir.AxisListType.X,
                                    op=mybir.AluOpType.add)
            limp = smallp.tile([1, 1], F32, name="limp_sb")
            nc.vector.tensor_mul(scr[:], st[:], ct[:])
            nc.vector.tensor_reduce(limp[:], scr[:], axis=mybir.AxisListType.X,
                                    op=mybir.AluOpType.add)
            den = smallp.tile([1, 1], F32, name="den_sb")
            nc.vector.tensor_add(den[:], gimp[:], limp[:])
            dinv = smallp.tile([1, 1], F32, name="dinv_sb")
            nc.vector.reciprocal(dinv[:], den[:])
            coef = smallp.tile([1, 1], F32, name="coef_sb")
            nc.vector.tensor_mul(coef[:], limp[:], dinv[:])

            if stage < 81:
                raise _StageDone()
            vcol = smallp.tile([128, 2], F32, name="vcol_sb")
            nc.vector.tensor_sub(vcol[:], memtc[:], stc[:])
            p_dm = ps_t.tile([32, 1], F32, name="p_dm", tag="pst")
            for hh in range(2):
                nc.tensor.matmul(p_dm[:], wu32[:, 32 * hh:32 * (hh + 1)],
                                 vcol[:, hh:hh + 1],
                                 start=(hh == 0), stop=(hh == 1))
            if stage < 82:
                raise _StageDone()
            coef_b = smallp.tile([32, 1], F32, name="coef_b")
            nc.gpsimd.partition_broadcast(coef_b[:], coef[:])
            d32 = smallp.tile([32, 1], F32, name="d32_sb")
            nc.vector.tensor_scalar_mul(d32[:], p_dm[:], coef_b[:])
            nv32 = smallp.tile([32, 1], F32, name="nv32_sb")
            nc.vector.tensor_add(nv32[:], memt32[:], d32[:])

            if stage < 83:
                raise _StageDone()
            nc.sync.dma_start(out=ct_o, in_=ct[:])
            nc.sync.dma_start(out=rt_o, in_=rt[:])
            if stage < 9:
                raise _StageDone()
            # patch the updated slot into x1s block d=1 (which holds my 32
            # rows of the grid at x1s[32+c, 66*(a+1)+b]), then write out
            pp = 66 * (slot // 64 + 1) + (slot % 64)
            nc.vector.tensor_copy(x1s[32:64, pp:pp + 1], nv32[:])
            x1s_3d = x1s[:].rearrange("p (r q) -> p r q", q=66)
            nc.sync.dma_start(out=nm_o.rearrange("p (a b) -> p a b", b=64),
                              in_=x1s_3d[32:64, 1:65, 0:64])
      except _StageDone:
        pass

    nc.compile()
    return nc


def shard_inputs(inputs, memory, conv_kernel1, conv_kernel2, conv_dense1,
                 conv_dense2, context_kernel, rec, slot):
    """Build the 8 per-core input dicts (host-side tiling/sharding)."""
    f32 = np.float32
    inputs = np.asarray(inputs, f32)
    memory = np.asarray(memory, f32)
    ck1 = np.asarray(conv_kernel1, f32)
    ck2 = np.asarray(conv_kernel2, f32)
    d1 = np.asarray(conv_dense1, f32)
    d2 = np.asarray(conv_dense2, f32)
    ctx = np.asarray(context_kernel, f32)
    rec = np.asarray(rec, f32)


    mem0 = np.ascontiguousarray(
        memory[:2048].reshape(16, 128, 256).transpose(1, 0, 2).reshape(128, 4096))
    mem1 = np.ascontiguousarray(
        memory[2048:].reshape(16, 128, 256).transpose(1, 0, 2).reshape(128, 4096))
    ctx_t = np.ascontiguousarray(
        ctx.reshape(3, 128, 256).transpose(1, 0, 2).reshape(128, 768))
    wu_t = np.ascontiguousarray(
        rec[128:].reshape(2, 128, 256).transpose(1, 0, 2).reshape(128, 512))
    id8 = np.eye(8, dtype=f32)
    id1 = np.ones((1, 1), f32)
    ones_col = np.ones((128, 1), f32)
    id128 = np.eye(128, dtype=f32)
    inp_col = np.ascontiguousarray(inputs.reshape(128, 1))
    memt_col = np.ascontiguousarray(memory[slot].reshape(2, 128).T)

    in_maps = []
    for i in range(N_CORES):
        x1p = np.zeros((32, 66, 66), f32)
        x1p[:, 1:65, 1:65] = memory[:, 32 * i:32 * (i + 1)].T.reshape(32, 64, 64)
        x1p = x1p.reshape(32, 4356)
        x1s = np.zeros((96, 4356), f32)
        for d in range(3):
            x1s[d * 32:(d + 1) * 32, 0:4356 - d] = x1p[:, d:]

        k1 = np.ascontiguousarray(
            ck1[:, :, 32 * i:32 * (i + 1), :].transpose(1, 2, 0, 3).reshape(96, 96))
        k2 = np.ascontiguousarray(
            ck2[:, :, :, 8 * i:8 * (i + 1)].transpose(1, 2, 0, 3).reshape(96, 24))

        sl = d1[8 * i * P61:8 * (i + 1) * P61]          # (8*3721, 128)
        slp = np.zeros((CH_PER_CORE, PPAD, 128), f32)   # pad to 3840/channel
        slp[:, :P61] = sl.reshape(CH_PER_CORE, P61, 128)
        # flat block j = t*8 + c holds positions 128t..128t+128 of channel c.
        # Diagonal-batch layout: group g packs blocks 4g..4g+4 side by side
        # so one N=512 matmul with lhsT = flat[:, 4g:4g+4] computes all four
        # (the useful outputs sit on the diagonal out[c, 128c+m]).
        slp = slp.reshape(CH_PER_CORE, NT, 128, 128).transpose(1, 0, 2, 3)
        d1main = np.ascontiguousarray(
            slp.reshape(NBLK // 4, 4, 128, 128)
            .transpose(0, 2, 1, 3).reshape(NBLK // 4, 128, 512))

        wu32 = np.ascontiguousarray(
            rec[128:].reshape(2, 128, 256)[:, :, 32 * i:32 * (i + 1)]
            .transpose(1, 0, 2).reshape(128, 64))
        memt32 = np.ascontiguousarray(
            memory[slot, 32 * i:32 * (i + 1)].reshape(32, 1))

        in_maps.append({
            "x1s": x1s, "k1": k1, "k2": k2, "id8": id8, "id128": id128,
            "id1": id1, "ones_col": ones_col,
            "inp_col": inp_col, "d1": d1main,
            "d2": d2, "ctxk": ctx_t, "wk": np.ascontiguousarray(rec[:128]),
            "wu": wu_t, "wu32": wu32, "memt_col": memt_col, "memt32": memt32,
            "mem0": mem0, "mem1": mem1,
        })
    return in_maps


_CACHE = {}


def _install_ntff_hook():
    """Register the axon NTFF profiling hook that this image's boot skips
    (its `antenv` package lacks `axon_hooks`). Needed only for trace runs."""
    import sys
    import types
    if "antenv.axon_hooks" in sys.modules:
        return
    mod = types.ModuleType("antenv.axon_hooks")
    mod._hook = None

    def set_axon_ntff_profile_hook(h):
        mod._hook = h

    def get_axon_ntff_profile_hook():
        return mod._hook

    mod.set_axon_ntff_profile_hook = set_axon_ntff_profile_hook
    mod.get_axon_ntff_profile_hook = get_axon_ntff_profile_hook
    sys.modules["antenv.axon_hooks"] = mod
    try:
        import antenv
        antenv.axon_hooks = mod
    except Exception:
        pass
    try:
        from trn_agent_boot.trn_boot import _ntff_profile_via_ctypes
        mod.set_axon_ntff_profile_hook(
            _ntff_profile_via_ctypes("/opt/axon/libaxon_pjrt.so"))
    except Exception:
        pass
    # the trace path tries to upload artifacts to S3; keep it local
    bass_utils.upload_artifacts = lambda d: str(d)


def kernel(inputs, memory, conv_kernel1, conv_kernel2, conv_dense1,
           conv_dense2, context_kernel, recurrent_kernel=None,
           recurr_kernel=None, x=32, y=17, _want_trace=False):
    rec = recurr_kernel if recurr_kernel is not None else recurrent_kernel
    slot = int(x) * 64 + int(y)

    if slot not in _CACHE:
        _CACHE[slot] = build_program(slot)
    nc = _CACHE[slot]

    in_maps = shard_inputs(inputs, memory, conv_kernel1, conv_kernel2,
                           conv_dense1, conv_dense2, context_kernel, rec, slot)

    if _want_trace:
        _install_ntff_hook()
    res = bass_utils.run_bass_kernel_spmd(
        nc, in_maps, core_ids=list(range(N_CORES)), trace=_want_trace,
    )
    c_t = np.asarray(res.results[0]["c_t"])
    r_t = np.asarray(res.results[0]["r_t"])
    new_mem = np.concatenate(
        [np.asarray(res.results[i]["new_mem_chunk"]) for i in range(N_CORES)],
        axis=0).reshape(UNITS, H, W)
    out = (c_t, r_t, new_mem)
    if _want_trace:
        return out, res
    return out
